# revision 34
# baseline (speedup 1.0000x reference)
"""DuelingDQN forward for 8 Trainium2 NeuronCores — pure batch data-parallel.

Per batch element b (reference semantics):
  market = state[b, :, :64]; port = state[b, 179, 64:]
  Q_h = market @ Wq_h.T + bq_h ; K_h likewise          (4 heads of 16)
  E_h = Q_h @ K_h.T / 4 ; P_h = softmax(E_h, -1)       (|E| small: skip max-sub)
  att = concat_h(mean_qs(P_h) @ V_h) @ Wo.T + bo       (V_h = market @ Wv_h.T)
  combined = [att, port] -> MLP dueling head -> out [3]

Hardware restructurings:
  * mean over query rows commutes with P@V:
      att_h = (mean_qs P_h) @ V_h
    so P@V collapses into a weighted column-sum of exp(E) done on the PE
    (softmax reciprocal rides in the moving operand), and V is never built:
      mbar = meanP @ market  (k-major, natural layouts), att = mbar @ Wv_h.T
  * Wo/bo folded into W1 host-side; all weight layout prep host-side.
  * q/k bias handled exactly via an appended constant-1 feature row.
  * qs rows packed 32-granular across pairs of batch elements so PE/ACT
    instructions run with full 128 partitions (2 b -> 3 full E-tiles).
  * bf16 on every high-volume PE path (fp32 matmul is 4 cyc/row, bf16 is 1).
"""

from contextlib import ExitStack

import numpy as np

S, F, MKT, H, HD, ATT = 180, 68, 64, 4, 16, 64
FC1, FC2, NACT = 256, 128, 3
B_TOT, NCORES = 2048, 8
BC = B_TOT // NCORES

_CACHE = {}


def _bf16(x):
    import ml_dtypes
    return np.asarray(x, np.float32).astype(ml_dtypes.bfloat16)


def _group_masks():
    """r-column masks for one 2-batch group (3 E-tiles).

    rbuf layout per group (16 cols, broadcast over 4 heads):
      0:4   tile0, b-even rows (all 128 real)
      4:8   tile1, b-even rows (0:52)
      8:12  tile2, b-odd rows (0:116)
      12:16 tile1, b-odd rows (64:128)
    """
    m0 = np.ones((128, 4), np.float32)
    m1e = np.zeros((128, 4), np.float32); m1e[0:52] = 1
    m2o = np.zeros((128, 4), np.float32); m2o[0:116] = 1
    m1o = np.zeros((128, 4), np.float32); m1o[64:128] = 1
    return np.concatenate([m0, m1e, m2o], 1), m1o


def _host_prep(inp):
    f32 = lambda x: np.ascontiguousarray(x, np.float32)
    Wq, Wk, Wv, Wo = (np.asarray(inp[k], np.float32) for k in ("Wq", "Wk", "Wv", "Wo"))
    bq, bk, bo, bv = (np.asarray(inp[k], np.float32) for k in ("bq", "bk", "bo", "bv"))

    # Stationary for Q/K projection: [65, 128].  K-dim = 64 market features
    # + one constant-1 row carrying the bias.  M-dim = 4 heads x 32 (16 real
    # dims + 16 zero pad so each head sits on a 32-aligned partition block).
    lq = np.zeros((MKT + 1, 128), np.float32)
    lk = np.zeros((MKT + 1, 128), np.float32)
    for h in range(H):
        lq[:MKT, 32 * h:32 * h + HD] = Wq[HD * h:HD * h + HD, :].T
        lq[MKT, 32 * h:32 * h + HD] = bq[HD * h:HD * h + HD]
        lk[:MKT, 32 * h:32 * h + HD] = Wk[HD * h:HD * h + HD, :].T
        lk[MKT, 32 * h:32 * h + HD] = bk[HD * h:HD * h + HD]

    W1, b1 = np.asarray(inp["W1"], np.float32), np.asarray(inp["b1"], np.float32)
    W1a, W1p = W1[:, :ATT], W1[:, ATT:]
    W1e = (W1a @ Wo).T                                         # [64, 256]
    # att rows live at partition 32h+d (16 real + 16 zero pad per head) so the
    # per-head copies land on 32-aligned partition bases.
    W1cT = np.zeros((128, FC1), np.float32)
    for h in range(H):
        W1cT[32 * h:32 * h + HD] = W1e[HD * h:HD * h + HD]
    W1pT = f32(W1p.T)                                          # [4, 256]
    # bv enters att exactly: the softmax-mean weights sum to 1, so
    # att = mbar @ Wv.T + bv  ->  fold (bo + Wo @ bv) through W1a.
    b1eff = b1 + W1a @ (bo + Wo @ bv)
    b1c = f32(b1eff.reshape(2, 128).T)                         # [128, 2]

    def noisy(p):
        W = inp[f"{p}_wmu"] + inp[f"{p}_wsig"] * inp[f"{p}_weps"]
        b = inp[f"{p}_bmu"] + inp[f"{p}_bsig"] * inp[f"{p}_beps"]
        return np.asarray(W, np.float32), np.asarray(b, np.float32)

    v1W, v1b = noisy("v1"); v2W, v2b = noisy("v2")
    a1W, a1b = noisy("a1"); a2W, a2b = noisy("a2")
    mask_a, mask_b = _group_masks()

    consts = {
        "lq": _bf16(lq), "lk": _bf16(lk),
        # 1/S folds the mean-over-query-positions into the Wv application
        "WvT": f32(Wv.T / S),
        "W1cT": f32(W1cT), "W1pT": W1pT, "b1c": b1c,
        "v1T": f32(v1W.T.reshape(2, 128, FC2).transpose(1, 0, 2)),  # [128,2,128]
        "a1T": f32(a1W.T.reshape(2, 128, FC2).transpose(1, 0, 2)),
        "v2T": f32(v2W.T), "a2T": f32(a2W.T),
        "bv1": f32(v1b.reshape(FC2, 1)), "ba1": f32(a1b.reshape(FC2, 1)),
        # v2 bias folded in: out = adv + (v - mean(adv)) + bv2 + (ba2 - mean(ba2))
        "ba2c": f32((a2b - a2b.mean() + v2b.reshape(-1)[0]).reshape(NACT, 1)),
        "ident": f32(np.eye(128)),
        "ones3": f32(np.full((NACT, 1), 1.0 / 3.0)),
        "mask_a": _bf16(np.tile(mask_a, (1, 4))),              # [128, 48]
        "mask_b": _bf16(np.tile(mask_b, (1, 4))),              # [128, 16]
    }
    return consts, float(v2b.reshape(-1)[0])


# E-tile slot layout per 2-batch group: slot -> (b offset in pair, qs block)
TILES_DEF = [
    [(0, 0), (0, 1), (0, 2), (0, 3)],
    [(0, 4), (0, 5), (1, 0), (1, 1)],
    [(1, 2), (1, 3), (1, 4), (1, 5)],
]
# colsum contributions: (tile_idx, rbuf col base, start, stop) per b-in-pair
CONTRIB = {0: [(0, 0, True, False), (1, 4, False, True)],
           1: [(1, 12, True, False), (2, 8, False, True)]}


def build_nc(bc=BC, nb=32, v2b=0.0, stage=99, nch_limit=None):
    import concourse.bacc as bacc
    import concourse.tile as tile
    from concourse import mybir

    fp32 = mybir.dt.float32
    bf16 = mybir.dt.bfloat16
    AF = mybir.ActivationFunctionType
    ALU = mybir.AluOpType
    AX = mybir.AxisListType

    assert nb % 8 == 0 and bc % nb == 0
    nch, ngrp = bc // nb, nb // 2
    nch_run = nch if nch_limit is None else min(nch, nch_limit)
    SP2 = 192      # per-b column stride in mktT/qT/kT: 180 real + 12 junk.
    # 192*2B keeps every per-b xbar-transpose output 32B-aligned, and the
    # 6th 32-wide qs block (cols 160:192) self-contains its junk pad.
    ncols = nb * SP2

    nc = bacc.Bacc(None, target_bir_lowering=False)
    st = nc.dram_tensor("state_c", [bc, S, F], fp32, kind="ExternalInput")
    out_d = nc.dram_tensor("out_c", [bc, NACT], fp32, kind="ExternalOutput")

    cshape = {
        "lq": ([MKT + 1, 128], bf16), "lk": ([MKT + 1, 128], bf16),
        "WvT": ([ATT, ATT], fp32),
        "W1cT": ([128, FC1], fp32), "W1pT": ([4, FC1], fp32),
        "b1c": ([128, 2], fp32),
        "v1T": ([128, 2, FC2], fp32), "a1T": ([128, 2, FC2], fp32),
        "v2T": ([FC2, 1], fp32), "a2T": ([FC2, NACT], fp32),
        "bv1": ([FC2, 1], fp32), "ba1": ([FC2, 1], fp32),
        "ba2c": ([NACT, 1], fp32),
        "ident": ([128, 128], fp32), "ones3": ([NACT, 1], fp32),
        "mask_a": ([128, 48], bf16), "mask_b": ([128, 16], bf16),
    }
    dts = {k: nc.dram_tensor(k, shp, dt, kind="ExternalInput")
           for k, (shp, dt) in cshape.items()}

    with tile.TileContext(nc) as tc, ExitStack() as ctx:
        constp = ctx.enter_context(tc.tile_pool(name="const", bufs=1))
        mkp = ctx.enter_context(tc.tile_pool(name="mk", bufs=2))
        mkbp = ctx.enter_context(tc.tile_pool(name="mkb", bufs=2))
        mktp = ctx.enter_context(tc.tile_pool(name="mktT", bufs=2))
        qktp = ctx.enter_context(tc.tile_pool(name="qkT", bufs=2))
        expp = ctx.enter_context(tc.tile_pool(name="expE", bufs=14))
        rsp = ctx.enter_context(tc.tile_pool(name="rs", bufs=3))
        smallp = ctx.enter_context(tc.tile_pool(name="small", bufs=4))
        # one shared PSUM pool: 2 slots x 4 banks.  Different heads' energy
        # matmuls must write DIFFERENT psum banks (same-bank same-col-group
        # concurrent PE writes crash the hardware), hence the 512 h-stride.
        psp = ctx.enter_context(tc.tile_pool(name="ps", bufs=2, space="PSUM"))

        cst = {}
        for k, (shp, dt) in cshape.items():
            t = constp.tile(shp, dt, tag=k, name=k + "_sb")
            nc.sync.dma_start(out=t[:], in_=dts[k][:])
            cst[k] = t
        outT = constp.tile([NACT, bc], fp32, tag="outT")
        nc.vector.memset(outT[:], 0.0)

        for ch in range(nch_run):
            b0 = ch * nb
            # ---------------- state load (s-major) ----------------------
            # two overlapping s-tiles: s 0:128 and s 116:180 (both 16-mult
            # partition counts, all slices start at partition 0)
            mk0 = mkp.tile([128, nb, F], fp32, tag="mk0")
            mk1 = mkp.tile([64, nb, F], fp32, tag="mk1")
            nc.sync.dma_start(
                out=mk0[:], in_=st[b0:b0 + nb, 0:128, :].transpose([1, 0, 2]))
            nc.sync.dma_start(
                out=mk1[:], in_=st[b0:b0 + nb, 116:180, :].transpose([1, 0, 2]))

            # ---------------- marketT via cast + DMA-xbar ----------------
            # staging: [s, b*128 + f] bf16, f-col 64 carries the constant 1
            mkb0 = mkbp.tile([128, nb, 128], bf16, tag="mkb0")
            mkb1 = mkbp.tile([64, nb, 128], bf16, tag="mkb1")
            nc.gpsimd.tensor_copy(mkb0[:, :, 0:MKT], mk0[:, :, 0:MKT])
            nc.gpsimd.memset(mkb0[:, :, MKT:], 0.0)
            nc.gpsimd.memset(mkb0[:, :, MKT:MKT + 1], 1.0)
            nc.gpsimd.tensor_copy(mkb1[:, :, 0:MKT], mk1[:, :, 0:MKT])
            nc.gpsimd.memset(mkb1[:, :, MKT:], 0.0)
            nc.gpsimd.memset(mkb1[:, :, MKT:MKT + 1], 1.0)

            mktT = mktp.tile([128, ncols], bf16, tag="mktT")
            mkview = mktT[:].rearrange("p (b c) -> p b c", c=SP2)
            nc.vector.memset(mkview[0:MKT + 1, :, 180:192], 0.0)
            mksc = mkbp.tile([128, nb, 64], bf16, tag="mksc")
            for b in range(nb):
                nc.sync.dma_start(out=mktT[:, b * SP2:b * SP2 + 128],
                                  in_=mkb0[:, b, :], transpose=True)
                # s 116:180 lands unaligned; bounce via aligned scratch
                nc.sync.dma_start(out=mksc[:, b, :],
                                  in_=mkb1[:, b, :], transpose=True)
            nc.vector.tensor_copy(mkview[0:128, :, 128:180], mksc[:, :, 12:64])

            # ---------------- Q/K projection -----------------------------
            qT = qktp.tile([128, ncols], bf16, tag="qT")
            kT = qktp.tile([128, ncols], bf16, tag="kT")
            for lhs, dst in ((cst["lq"], qT), (cst["lk"], kT)):
                c = 0
                while c < ncols:
                    pp = psp.tile([128, 1024], fp32, tag="ps")
                    w = min(1024, ncols - c)
                    for j in range(0, w, 512):
                        wj = min(512, w - j)
                        nc.tensor.matmul(
                            pp[:, j:j + wj], lhs[0:MKT + 1, :],
                            mktT[0:MKT + 1, c + j:c + j + wj],
                            start=True, stop=True)
                    nc.vector.tensor_copy(dst[:, c:c + w], pp[:, 0:w])
                    c += w

            # ---------------- attention ---------------------------------
            if stage < 2:
                continue
            for gq in range(ngrp // 4):          # 4 pair-groups = 8 b
                expts = {}
                rsum = rsp.tile([128, 48], bf16, tag="rsum")
                for gl in range(4):
                    g = 4 * gq + gl
                    for t3 in range(3):
                        ep = psp.tile([128, 2048], fp32, tag="ps")
                        for sl in range(4):
                            db, qb = TILES_DEF[t3][sl]
                            b = 2 * g + db
                            qc = b * SP2 + 32 * qb
                            for h in range(H):
                                nc.tensor.matmul(
                                    ep[32 * sl:32 * sl + 32,
                                       512 * h:512 * h + 180],
                                    qT[32 * h:32 * h + 32, qc:qc + 32],
                                    kT[32 * h:32 * h + 32,
                                       b * SP2:b * SP2 + 180],
                                    start=True, stop=True,
                                    tile_position=(32 * h, 32 * sl))
                        ex = expp.tile([128, H * S], bf16, tag="expE")
                        nc.scalar.activation(
                            ex[:],
                            ep[:].rearrange("p (h x) -> p h x", h=H)[:, :, 0:180],
                            AF.Exp, scale=0.25)
                        expts[(gl, t3)] = ex
                        with nc.allow_low_precision(reason="softmax denom bf16"):
                            nc.vector.tensor_reduce(
                                rsum[:, 12 * gl + 4 * t3:12 * gl + 4 * t3 + 4],
                                ex[:].rearrange("p (h s) -> p h s", h=H),
                                axis=AX.X, op=ALU.add)
                if stage < 3:
                    continue
                # r = mask / rowsum (bf16), batched over 4 pair-groups
                rec = rsp.tile([128, 48], bf16, tag="rec")
                rbuf = rsp.tile([128, 4, 16], bf16, tag="rbuf")
                with nc.allow_low_precision(reason="softmax recip bf16"):
                    nc.vector.reciprocal(rec[:], rsum[:])
                rec3 = rec[:].rearrange("p (g x) -> p g x", g=4)
                nc.vector.tensor_tensor(
                    out=rbuf[:, :, 0:12], in0=rec3, in1=cst["mask_a"][:],
                    op=ALU.mult)
                nc.vector.tensor_tensor(
                    out=rbuf[:, :, 12:16], in0=rec3[:, :, 4:8],
                    in1=cst["mask_b"][:], op=ALU.mult)

                if stage < 4:
                    continue
                # weighted column sums -> meanPT [ks, (b8, h, kst)]
                # (each accumulation group start/stop pair emitted adjacently:
                #  the psum model allows one pending group per zero region)
                mpt = psp.tile([128, 64], fp32, tag="ps", name="mpt")
                for gl in range(4):
                    for db in range(2):
                        b8 = 2 * gl + db
                        for h in range(H):
                            for kst, (c0, cw) in enumerate(
                                    ((0, 116), (116, 64))):
                                oc = 8 * b8 + 2 * h + kst
                                for (t3, rcb, sa, so) in CONTRIB[db]:
                                    nc.tensor.matmul(
                                        mpt[0:cw, oc:oc + 1],
                                        expts[(gl, t3)][:, S * h + c0:
                                                        S * h + c0 + cw],
                                        rbuf[:, gl, rcb + h:rcb + h + 1],
                                        start=sa, stop=so)
                mptS = smallp.tile([128, 8, 4, 2], fp32, tag="mptS")
                mptv = mpt[:].rearrange("p (b h t) -> p b h t", b=8, h=H)
                nc.vector.tensor_copy(mptS[0:116, :, :, 0], mptv[0:116, :, :, 0])
                nc.vector.tensor_copy(mptS[0:64, :, :, 1], mptv[0:64, :, :, 1])

                if stage < 5:
                    continue
                # mbarT[mkt, 4h] per b = market_b^T @ meanPT_b
                SP = psp.tile([128, 512], fp32, tag="ps", name="SP")
                for b8 in range(8):
                    b = 8 * gq + b8
                    nc.tensor.matmul(SP[0:MKT, 4 * b8:4 * b8 + 4],
                                     mk0[0:116, b, 0:MKT], mptS[0:116, b8, :, 0],
                                     start=True, stop=False)
                    nc.tensor.matmul(SP[0:MKT, 4 * b8:4 * b8 + 4],
                                     mk1[:, b, 0:MKT], mptS[0:64, b8, :, 1],
                                     start=False, stop=True)
                mbS = smallp.tile([MKT, 32], fp32, tag="mbS")
                nc.vector.tensor_copy(mbS[:], SP[0:MKT, 0:32])

                # att_pre per head -> comb rows 32h:32h+16 (32-aligned bases)
                for h in range(H):
                    for b8 in range(8):
                        nc.tensor.matmul(
                            SP[32 * h:32 * h + HD, 64 + b8:64 + b8 + 1],
                            cst["WvT"][:, HD * h:HD * h + HD],
                            mbS[:, 4 * b8 + h:4 * b8 + h + 1],
                            start=True, stop=True, tile_position=(0, 32 * h))
                comb = smallp.tile([128, 8], fp32, tag="comb")
                nc.vector.memset(comb[:], 0.0)
                for h in range(H):
                    nc.vector.tensor_copy(
                        comb[32 * h:32 * h + HD, :],
                        SP[32 * h:32 * h + HD, 64:72])
                portT = smallp.tile([4, 8], fp32, tag="portT")
                nc.sync.dma_start(
                    out=portT[:].rearrange("p (b o) -> p b o", o=1),
                    in_=st[b0 + 8 * gq:b0 + 8 * gq + 8, 179:180, ATT:F
                           ].transpose([2, 0, 1]))

                if stage < 6:
                    continue
                # ---- MLP dueling head (8 columns) -----------------------
                for hf in range(2):
                    nc.tensor.matmul(SP[:, 96 + 8 * hf:104 + 8 * hf],
                                     cst["W1cT"][:, 128 * hf:128 * hf + 128],
                                     comb[:], start=True, stop=False)
                    nc.tensor.matmul(SP[:, 96 + 8 * hf:104 + 8 * hf],
                                     cst["W1pT"][:, 128 * hf:128 * hf + 128],
                                     portT[:], start=False, stop=True)
                ft = smallp.tile([128, 2, 8], fp32, tag="ft")
                for hf in range(2):
                    nc.scalar.activation(
                        ft[:, hf, :], SP[:, 96 + 8 * hf:104 + 8 * hf],
                        AF.Relu, bias=cst["b1c"][:, hf:hf + 1])
                for hi, w1t in ((0, "v1T"), (1, "a1T")):
                    for hf in range(2):
                        nc.tensor.matmul(SP[:, 112 + 8 * hi:120 + 8 * hi],
                                         cst[w1t][:, hf, :], ft[:, hf, :],
                                         start=(hf == 0), stop=(hf == 1))
                ht = smallp.tile([128, 2, 8], fp32, tag="ht")
                for hi, bvec in ((0, "bv1"), (1, "ba1")):
                    nc.scalar.activation(
                        ht[:, hi, :], SP[:, 112 + 8 * hi:120 + 8 * hi],
                        AF.Relu, bias=cst[bvec][:])
                nc.tensor.matmul(SP[0:1, 128:136], cst["v2T"][:], ht[:, 0, :],
                                 start=True, stop=True)
                nc.tensor.matmul(SP[0:NACT, 136:144], cst["a2T"][:],
                                 ht[:, 1, :], start=True, stop=True)
                adv = smallp.tile([NACT, 8], fp32, tag="adv")
                nc.vector.tensor_copy(adv[:], SP[0:NACT, 136:144])
                nc.tensor.matmul(SP[0:1, 144:152], cst["ones3"][:], adv[:],
                                 start=True, stop=True)
                vm = smallp.tile([1, 24], fp32, tag="vm")
                nc.vector.tensor_copy(vm[:], SP[0:1, 128:152])
                w = smallp.tile([1, 8], fp32, tag="w")
                nc.vector.tensor_tensor(out=w[:], in0=vm[:, 0:8],
                                        in1=vm[:, 16:24], op=ALU.subtract)
                w3 = smallp.tile([NACT, 8], fp32, tag="w3")
                nc.gpsimd.partition_broadcast(w3[:], w[:], channels=NACT)
                o1 = smallp.tile([NACT, 8], fp32, tag="o1")
                nc.vector.tensor_tensor(out=o1[:], in0=adv[:], in1=w3[:],
                                        op=ALU.add)
                nc.vector.tensor_scalar(
                    out=outT[:, b0 + 8 * gq:b0 + 8 * gq + 8], in0=o1[:],
                    scalar1=cst["ba2c"][:], scalar2=None, op0=ALU.add)

        # ---------------- store output ------------------------------------
        for half in range((bc + 127) // 128):
            wbc = min(128, bc - 128 * half)
            op = psp.tile([128, 512], fp32, tag="ps", name="op")
            nc.tensor.transpose(op[0:wbc, 0:NACT],
                                outT[:, 128 * half:128 * half + wbc],
                                cst["ident"][0:NACT, 0:NACT])
            os_ = smallp.tile([128, NACT], fp32, tag="os")
            nc.vector.tensor_copy(os_[0:wbc, :], op[0:wbc, 0:NACT])
            nc.sync.dma_start(out=out_d[128 * half:128 * half + wbc, :],
                              in_=os_[0:wbc, :])

    nc.compile()
    return nc


def run_raw(inputs, **spmd_kwargs):
    if "nc" not in _CACHE:
        consts, v2b = _host_prep(inputs)
        _CACHE["consts"] = consts
        _CACHE["nc"] = build_nc(BC, 32, v2b)
    consts, nc = _CACHE["consts"], _CACHE["nc"]

    from concourse.bass_utils import run_bass_kernel_spmd
    state = np.ascontiguousarray(inputs["state"], np.float32)
    in_maps = [dict(consts, state_c=np.ascontiguousarray(state[c * BC:(c + 1) * BC]))
               for c in range(NCORES)]
    return run_bass_kernel_spmd(nc, in_maps, core_ids=list(range(NCORES)),
                                **spmd_kwargs)


def kernel(**inputs):
    res = run_raw(inputs)
    return np.concatenate(
        [res.results[c]["out_c"] for c in range(NCORES)], axis=0).astype(np.float32)
